# revision 2
# baseline (speedup 1.0000x reference)
"""Block-sparse attention Trainium2 kernel (8 NeuronCores, SPMD).

Problem: hidden_states [2, 2048, 2048] fp32; Wq/Wk/Wv [2048, 2048]; Wo
[2048, 2048]. 16 heads x 128 dim, block-banded attention (BLOCK=64,
bandwidth 2 -> each 128-query tile attends a 384-key band with two
64x64 invalid corners).

Sharding: core c = (batch b = c//4) x (head group g = c%4, 4 heads).
Each core computes q/k/v projections for its 4 heads, banded
attention, and a partial output through its rows of Wo. Host sums the
4 partials per batch. No collectives.

Host-side packing (all bf16):
  htq  [8192, 512]  = h^T stacked as 4 contiguous seq-quarters
                      (rows 2048*q + hid, cols = seq quarter q)
  wqkv [2048, 1536] = [Wq | Wk | Wv] column-block for this head group
  wo   [512, 2048]  = Wo rows for this head group

DMA plan (arrival order matched to consumption; contiguous 128KB
tiles let SWDGE aggregate packets):
  gpsimd SW : htq q0 (16), htq q1 evens (8), htq q3 (16), wo (4)
  sync  HW  : wqkv odds (8), htq q1 odds (8), then AO dma-transposes
  scalar HW : wqkv evens (8), htq q2 (16), then output stores

Compute: warm-up matmuls cover the ~9us DMA latency; then a "chase"
phase of 6 PSUM groups (V t0-3 + Q/K head0 mc0) consumes k-tiles in
arrival order so the PE never idles long enough for HAM to demote the
clock. Attention is computed transposed: S^T blocks via lhsT=KT,
exp straight from PSUM (corner masking via gpsimd memsets of P^T),
PV with lhsT=P^T produces AO in natural [q, d] layout with a fused
row-sum column (V tiles carry a ones column), per-partition
normalize, then DMA-transpose AO -> AO^T tiles for the Wo matmuls
(no PE transposes at all). Wo is fused into head 3's loop at lag 1.
"""

from contextlib import ExitStack

import numpy as np

import concourse.bass as bass
import concourse.mybir as mybir
import concourse.tile as tile
from concourse import bacc
from concourse.bass_utils import run_bass_kernel_spmd
from concourse.masks import make_identity

S = 2048          # sequence length
HID = 2048        # hidden size
HL = 4            # heads per core
D = 128           # head dim
NKT = HID // 128  # 16 contraction tiles
NQ = S // 128     # 16 query tiles
SCALE = float(D) ** -0.5
BF = mybir.dt.bfloat16
F32 = mybir.dt.float32
N_WARM = 72


def _emit_wo(nc, ps_big, osb_pool, aot, wo_t, out, mt):
    osb = osb_pool.tile([128, HID], BF, tag="osb", name="osb")
    for nc_ in range(4):
        ns = slice(512 * nc_, 512 * (nc_ + 1))
        ops_ = ps_big.tile([128, 512], F32, tag="big", name="wops")
        for dk in range(HL):
            nc.tensor.matmul(
                ops_, lhsT=aot[dk][mt], rhs=wo_t[dk][:, ns],
                start=(dk == 0), stop=(dk == HL - 1),
            )
        nc.any.tensor_copy(osb[:, ns], ops_)
    nc.scalar.dma_start(out=out[128 * mt : 128 * (mt + 1), :], in_=osb)


def build():
    nc = bacc.Bacc()
    htq = nc.declare_dram_parameter("htq", [4 * HID, 512], BF, isOutput=False)
    wqkv = nc.declare_dram_parameter("wqkv", [HID, 3 * HL * D], BF, isOutput=False)
    wo = nc.declare_dram_parameter("wo", [HL * D, HID], BF, isOutput=False)
    out = nc.declare_dram_parameter("out", [S, HID], BF, isOutput=True)

    with ExitStack() as ctx:
        tc = ctx.enter_context(tile.TileContext(nc))
        persist = ctx.enter_context(tc.tile_pool(name="persist", bufs=1))
        qk = ctx.enter_context(tc.tile_pool(name="qk", bufs=2))
        work = ctx.enter_context(tc.tile_pool(name="work", bufs=3))
        stats = ctx.enter_context(tc.tile_pool(name="stats", bufs=8))
        osb_pool = ctx.enter_context(tc.tile_pool(name="osb", bufs=2))
        ps_big = ctx.enter_context(tc.tile_pool(name="ps_big", bufs=6, space="PSUM"))
        ps_sc = ctx.enter_context(tc.tile_pool(name="ps_sc", bufs=1, space="PSUM"))
        ps_ao = ctx.enter_context(tc.tile_pool(name="ps_ao", bufs=1, space="PSUM"))

        ident = persist.tile([128, 128], BF, tag="ident")
        make_identity(nc, ident)

        # HAM warm-up: dependency-free matmuls from t=0 flip the PE clock
        # gate to 2.4GHz and cover the ~9us DMA startup latency.
        warm_ps = ps_ao.tile([128, 129], F32, tag="ao", name="warm_ps")
        for _ in range(N_WARM):
            nc.tensor.matmul(warm_ps[:, 0:128], lhsT=ident, rhs=ident, start=True, stop=True)

        # ---- input tiles
        htq_t = [
            [persist.tile([128, 512], BF, tag=f"ht{q}_{k}", name=f"ht{q}_{k}") for k in range(NKT)]
            for q in range(4)
        ]
        wqkv_t = [persist.tile([128, 3 * HL * D], BF, tag=f"wqkv{k}", name=f"wqkv{k}") for k in range(NKT)]
        wo_t = [persist.tile([128, HID], BF, tag=f"wo{d}", name=f"wo{d}") for d in range(HL)]

        def ld_ht(eng, q, k):
            eng.dma_start(out=htq_t[q][k], in_=htq[2048 * q + 128 * k : 2048 * q + 128 * (k + 1), :])

        # gpsimd SW stream: q0 all, q1 evens, q3 all, wo
        for k in range(NKT):
            ld_ht(nc.gpsimd, 0, k)
        # sync HW: wqkv odds first (chase pacing), scalar HW: wqkv evens
        for k in range(NKT):
            ks = slice(128 * k, 128 * (k + 1))
            eng = nc.sync if (k % 2) else nc.scalar
            eng.dma_start(out=wqkv_t[k], in_=wqkv[ks, :])
        for k in range(0, NKT, 2):
            ld_ht(nc.gpsimd, 1, k)
        for k in range(1, NKT, 2):
            ld_ht(nc.sync, 1, k)
        for k in range(NKT):
            ld_ht(nc.scalar, 2, k)
        for k in range(NKT):
            ld_ht(nc.gpsimd, 3, k)
        for d in range(HL):
            nc.gpsimd.dma_start(out=wo_t[d], in_=wo[128 * d : 128 * (d + 1), :])

        # V tiles: [128, head, 129]; col 128 of each head block is ones so
        # the PV matmul's last column accumulates the softmax row-sum.
        V_t = [persist.tile([128, HL, 129], BF, tag=f"v{t}", name=f"v{t}") for t in range(NQ)]
        for t in range(NQ):
            nc.gpsimd.memset(V_t[t][:, :, 128:129], 1.0)

        # AO^T tiles, one per (head, qt); written by DMA transpose.
        aot = [
            [persist.tile([128, 128], BF, tag=f"ao{h}_{qt}", name=f"ao{h}_{qt}") for qt in range(NQ)]
            for h in range(HL)
        ]

        def v_copy(t, vps):
            for h4 in range(HL):
                nc.any.tensor_copy(V_t[t][:, h4, 0:128], vps[:, 128 * h4 : 128 * (h4 + 1)])

        def qk_proj(h, mc, QT, KT):
            ms = slice(512 * mc, 512 * (mc + 1))
            qps = ps_big.tile([128, 512], F32, tag="big", name="qps")
            for k in range(NKT):
                nc.tensor.matmul(
                    qps, lhsT=wqkv_t[k][:, 128 * h : 128 * (h + 1)], rhs=htq_t[mc][k],
                    start=(k == 0), stop=(k == NKT - 1),
                )
            nc.vector.tensor_scalar_mul(QT[:, ms], qps, SCALE)
            kps = ps_big.tile([128, 512], F32, tag="big", name="kps")
            for k in range(NKT):
                nc.tensor.matmul(
                    kps, lhsT=wqkv_t[k][:, 512 + 128 * h : 512 + 128 * (h + 1)], rhs=htq_t[mc][k],
                    start=(k == 0), stop=(k == NKT - 1),
                )
            nc.vector.tensor_copy(KT[:, ms], kps)

        def v_proj(t):
            q, tl = divmod(t, 4)
            vps = ps_big.tile([128, 512], F32, tag="big", name="vps")
            for k in range(NKT):
                nc.tensor.matmul(
                    vps, lhsT=htq_t[q][k][:, 128 * tl : 128 * (tl + 1)], rhs=wqkv_t[k][:, 1024:1536],
                    start=(k == 0), stop=(k == NKT - 1),
                )
            v_copy(t, vps)

        def attention(h, QT, KT):
            for qt in range(NQ):
                t0 = max(0, 128 * qt - 128)
                t1 = min(S, 128 * qt + 256)
                nch = (t1 - t0) // 128
                # scores^T: one accumulation group, blocks at distinct cols
                scps = ps_sc.tile([128, 384], F32, tag="sc", name="scps")
                for b in range(nch):
                    nc.tensor.matmul(
                        scps[:, 128 * b : 128 * (b + 1)],
                        lhsT=KT[:, t0 + 128 * b : t0 + 128 * (b + 1)],
                        rhs=QT[:, 128 * qt : 128 * (qt + 1)],
                        start=(b == 0), stop=(b == nch - 1),
                        skip_group_check=True,
                    )
                # scores are O(+-8): exp needs no max subtraction (softmax is
                # shift-invariant; fp32 exp is safe here)
                p = work.tile([128, 384], BF, tag="p", name="p")
                nc.scalar.activation(
                    p[:, 0 : 128 * nch], scps[:, 0 : 128 * nch],
                    mybir.ActivationFunctionType.Exp, bias=0.0, scale=1.0,
                )
                # band corners are invalid: zero them in P^T instead of
                # adding -inf masks pre-exp
                if qt > 0:
                    nc.gpsimd.memset(p[0:64, 64:128], 0.0)
                if qt < NQ - 1:
                    nc.gpsimd.memset(p[64:128, 128 * (nch - 1) : 128 * (nch - 1) + 64], 0.0)
                aops = ps_ao.tile([128, 129], F32, tag="ao", name="aops")
                for b in range(nch):
                    tt = t0 // 128 + b
                    nc.tensor.matmul(
                        aops, lhsT=p[:, 128 * b : 128 * (b + 1)], rhs=V_t[tt][:, h, :],
                        start=(b == 0), stop=(b == nch - 1),
                    )
                rcp = stats.tile([128, 1], F32, tag="rcp", name="rcp")
                nc.vector.reciprocal(rcp, aops[:, 128:129])
                ao_sb = work.tile([128, 128], BF, tag="aosb", name="ao_sb")
                nc.vector.tensor_scalar_mul(ao_sb, aops[:, 0:128], rcp)
                nc.sync.dma_start_transpose(out=aot[h][qt], in_=ao_sb)
                if h == HL - 1 and qt >= 1:
                    _emit_wo(nc, ps_big, osb_pool, aot, wo_t, out, qt - 1)

        # ---- head 0: chase phase (V t0-3 + Q/K mc0 consume k-slices in
        # arrival order), then mc1-3 interleaved with V by quarter
        QT0 = qk.tile([128, S], BF, tag="q", name="qt0")
        KT0 = qk.tile([128, S], BF, tag="k", name="kt0")
        vps_c = [ps_big.tile([128, 512], F32, tag="big", name=f"vpsc{t}") for t in range(4)]
        qps_c = ps_big.tile([128, 512], F32, tag="big", name="qpsc")
        kps_c = ps_big.tile([128, 512], F32, tag="big", name="kpsc")
        for k in range(NKT):
            st, sp = (k == 0), (k == NKT - 1)
            nc.tensor.matmul(
                qps_c, lhsT=wqkv_t[k][:, 0:128], rhs=htq_t[0][k], start=st, stop=sp
            )
            nc.tensor.matmul(
                kps_c, lhsT=wqkv_t[k][:, 512:640], rhs=htq_t[0][k], start=st, stop=sp
            )
            for t in range(4):
                nc.tensor.matmul(
                    vps_c[t], lhsT=htq_t[0][k][:, 128 * t : 128 * (t + 1)],
                    rhs=wqkv_t[k][:, 1024:1536], start=st, stop=sp,
                )
        nc.vector.tensor_scalar_mul(QT0[:, 0:512], qps_c, SCALE)
        nc.vector.tensor_copy(KT0[:, 0:512], kps_c)
        for t in range(4):
            v_copy(t, vps_c[t])

        # quarter-paced: mc1 + V t4-7 chase htq q1, mc2 + V t8-11 chase q2,
        # mc3 + V t12-15 chase q3
        for mc in (1, 2, 3):
            qk_proj(0, mc, QT0, KT0)
            for t in range(4 * mc, 4 * mc + 4):
                v_proj(t)
        attention(0, QT0, KT0)

        for h in range(1, HL):
            QT = qk.tile([128, S], BF, tag="q", name=f"qt{h}")
            KT = qk.tile([128, S], BF, tag="k", name=f"kt{h}")
            for mc in range(4):
                qk_proj(h, mc, QT, KT)
            attention(h, QT, KT)
        _emit_wo(nc, ps_big, osb_pool, aot, wo_t, out, NQ - 1)

    if not nc.is_finalized():
        nc.finalize()
    return nc


_NC = None


def _get_nc():
    global _NC
    if _NC is None:
        _NC = build()
    return _NC


def _in_maps(hidden_states, Wq, Wk, Wv, Wo):
    import ml_dtypes

    bf = ml_dtypes.bfloat16
    hs = np.asarray(hidden_states, dtype=np.float32)
    Wq = np.asarray(Wq, dtype=np.float32)
    Wk = np.asarray(Wk, dtype=np.float32)
    Wv = np.asarray(Wv, dtype=np.float32)
    Wo = np.asarray(Wo, dtype=np.float32)
    maps = []
    for c in range(8):
        b, g = divmod(c, 4)
        sl = slice(512 * g, 512 * (g + 1))
        hsT = hs[b].T  # [hid, seq]
        htq = np.concatenate([hsT[:, 512 * q : 512 * (q + 1)] for q in range(4)], axis=0)
        wqkv = np.concatenate([Wq[:, sl], Wk[:, sl], Wv[:, sl]], axis=1)
        maps.append(
            {
                "htq": np.ascontiguousarray(htq).astype(bf),
                "wqkv": np.ascontiguousarray(wqkv).astype(bf),
                "wo": np.ascontiguousarray(Wo[sl, :]).astype(bf),
            }
        )
    return maps


def _gather(results):
    outs = [np.asarray(results[c]["out"]).astype(np.float32) for c in range(8)]
    return np.stack(
        [outs[0] + outs[1] + outs[2] + outs[3],
         outs[4] + outs[5] + outs[6] + outs[7]]
    )


def run(in_maps, trace=False, **kw):
    nc = _get_nc()
    return run_bass_kernel_spmd(nc, in_maps, core_ids=list(range(8)), trace=trace, **kw)


def kernel(hidden_states, Wq, Wk, Wv, Wo):
    maps = _in_maps(hidden_states, Wq, Wk, Wv, Wo)
    res = run(maps)
    return _gather(res.results)


# revision 9
# speedup vs baseline: 1.0256x; 1.0256x over previous
"""Block-sparse attention Trainium2 kernel (8 NeuronCores, SPMD).

Problem: hidden_states [2, 2048, 2048] fp32; Wq/Wk/Wv [2048, 2048]; Wo
[2048, 2048]. 16 heads x 128 dim, block-banded attention (BLOCK=64,
bandwidth 2 -> each 128-query tile attends a 384-key band with two
64x64 invalid corners).

Sharding: core c = (batch b = c//4) x (head group g = c%4, 4 heads).
Each core computes q/k/v projections for its 4 heads, banded
attention, and a partial output through its rows of Wo. Host sums the
4 partials per batch. No collectives.

Host-side packing (all bf16):
  htq  [8192, 512]  = h^T stacked as 4 contiguous seq-quarters
                      (rows 2048*q + hid, cols = seq quarter q)
  wqkv [2048, 1536] = [Wq | Wk | Wv] column-block for this head group
  wo   [512, 2048]  = Wo rows for this head group

DMA plan (arrival order matched to consumption; contiguous 128KB
tiles let SWDGE aggregate packets):
  gpsimd SW : htq q0 (16), htq q1 evens (8), htq q3 (16), wo (4)
  sync  HW  : wqkv odds (8), htq q1 odds (8), then AO dma-transposes
  scalar HW : wqkv evens (8), htq q2 (16), then output stores

Compute: warm-up matmuls cover the ~9us DMA latency; then a "chase"
phase of 6 PSUM groups (V t0-3 + Q/K head0 mc0) consumes k-tiles in
arrival order so the PE never idles long enough for HAM to demote the
clock. Attention is computed transposed: S^T blocks via lhsT=KT,
exp straight from PSUM (corner masking via gpsimd memsets of P^T),
PV with lhsT=P^T produces AO in natural [q, d] layout with a fused
row-sum column (V tiles carry a ones column), per-partition
normalize, then DMA-transpose AO -> AO^T tiles for the Wo matmuls
(no PE transposes at all). Wo is fused into head 3's loop at lag 1.
"""

from contextlib import ExitStack

import numpy as np

import concourse.bass as bass
import concourse.mybir as mybir
import concourse.tile as tile
from concourse import bacc
from concourse.bass_utils import run_bass_kernel_spmd
from concourse.masks import make_identity

S = 2048          # sequence length
HID = 2048        # hidden size
HL = 4            # heads per core
D = 128           # head dim
NKT = HID // 128  # 16 contraction tiles
NQ = S // 128     # 16 query tiles
SCALE = float(D) ** -0.5
BF = mybir.dt.bfloat16
F32 = mybir.dt.float32
N_WARM = 84
WO_LAG = 2


def _emit_wo(nc, ps_big, osb_pool, aot, wo_t, out, mt):
    osb = osb_pool.tile([128, HID], BF, tag="osb", name="osb")
    for nc_ in range(4):
        ns = slice(512 * nc_, 512 * (nc_ + 1))
        ops_ = ps_big.tile([128, 512], F32, tag="big", name="wops")
        for dk in range(HL):
            nc.tensor.matmul(
                ops_, lhsT=aot[dk][mt], rhs=wo_t[dk][:, ns],
                start=(dk == 0), stop=(dk == HL - 1),
            )
        nc.any.tensor_copy(osb[:, ns], ops_)
    nc.scalar.dma_start(out=out[128 * mt : 128 * (mt + 1), :], in_=osb)


def build():
    nc = bacc.Bacc()
    htq = nc.declare_dram_parameter("htq", [2 * HID, 512], BF, isOutput=False)
    hth = nc.declare_dram_parameter("hth", [HID, 1024], BF, isOutput=False)
    wqkv = nc.declare_dram_parameter("wqkv", [HID, 3 * HL * D], BF, isOutput=False)
    wo = nc.declare_dram_parameter("wo", [HL * D, HID], BF, isOutput=False)
    out = nc.declare_dram_parameter("out", [S, HID], BF, isOutput=True)

    with ExitStack() as ctx:
        tc = ctx.enter_context(tile.TileContext(nc))
        persist = ctx.enter_context(tc.tile_pool(name="persist", bufs=1))
        qk = ctx.enter_context(tc.tile_pool(name="qk", bufs=2))
        work = ctx.enter_context(tc.tile_pool(name="work", bufs=3))
        stats = ctx.enter_context(tc.tile_pool(name="stats", bufs=8))
        osb_pool = ctx.enter_context(tc.tile_pool(name="osb", bufs=2))
        ps_big = ctx.enter_context(tc.tile_pool(name="ps_big", bufs=6, space="PSUM"))
        ps_sc = ctx.enter_context(tc.tile_pool(name="ps_sc", bufs=1, space="PSUM"))
        ps_ao = ctx.enter_context(tc.tile_pool(name="ps_ao", bufs=1, space="PSUM"))

        ident = persist.tile([128, 128], BF, tag="ident")
        make_identity(nc, ident)

        # HAM warm-up: dependency-free matmuls from t=0 flip the PE clock
        # gate to 2.4GHz and cover the ~9us DMA startup latency.
        warm_ps = ps_ao.tile([128, 129], F32, tag="ao", name="warm_ps")
        for _ in range(N_WARM):
            nc.tensor.matmul(warm_ps[:, 0:128], lhsT=ident, rhs=ident, start=True, stop=True)

        # ---- input tiles. Quarters 0/1 as [128,512] tiles on the SWDGE
        # stream (contiguous -> packet aggregation); the second seq half as
        # [128,1024] tiles (2KB rows) on the two HWDGE queues.
        htq_t = [
            [persist.tile([128, 512], BF, tag=f"ht{q}_{k}", name=f"ht{q}_{k}") for k in range(NKT)]
            for q in range(2)
        ]
        hb_t = [persist.tile([128, 1024], BF, tag=f"hb{k}", name=f"hb{k}") for k in range(NKT)]
        wqkv_t = [persist.tile([128, 3 * HL * D], BF, tag=f"wqkv{k}", name=f"wqkv{k}") for k in range(NKT)]
        wo_t = [persist.tile([128, HID], BF, tag=f"wo{d}", name=f"wo{d}") for d in range(HL)]

        def ht_rhs(mc, k):
            # moving operand for Q/K projections: h^T[:, 512*mc : 512*(mc+1)]
            if mc < 2:
                return htq_t[mc][k]
            return hb_t[k][:, 512 * (mc - 2) : 512 * (mc - 1)]

        def ht_col(t, k):
            # stationary operand for V projection: h^T cols 128*t : 128*(t+1)
            if t < 8:
                return htq_t[t // 4][k][:, 128 * (t % 4) : 128 * (t % 4) + 128]
            return hb_t[k][:, 128 * (t - 8) : 128 * (t - 8) + 128]

        for k in range(NKT):
            nc.gpsimd.dma_start(out=htq_t[0][k], in_=htq[128 * k : 128 * (k + 1), :])
        for k in range(NKT):
            ks = slice(128 * k, 128 * (k + 1))
            eng = nc.sync if (k % 2) else nc.scalar
            eng.dma_start(out=wqkv_t[k], in_=wqkv[ks, :])
        for k in range(NKT):
            nc.gpsimd.dma_start(out=htq_t[1][k], in_=htq[2048 + 128 * k : 2048 + 128 * (k + 1), :])
        for k in range(NKT):
            ks = slice(128 * k, 128 * (k + 1))
            eng = nc.sync if (k % 2) else nc.scalar
            eng.dma_start(out=hb_t[k], in_=hth[ks, :])
        for d in range(HL):
            nc.gpsimd.dma_start(out=wo_t[d], in_=wo[128 * d : 128 * (d + 1), :])

        # V tiles: [128, head, 129]; col 128 of each head block is ones so
        # the PV matmul's last column accumulates the softmax row-sum.
        V_t = [persist.tile([128, HL, 129], BF, tag=f"v{t}", name=f"v{t}") for t in range(NQ)]
        for t in range(NQ):
            nc.gpsimd.memset(V_t[t][:, :, 128:129], 1.0)

        # AO^T tiles, one per (head, qt); written by DMA transpose.
        aot = [
            [persist.tile([128, 128], BF, tag=f"ao{h}_{qt}", name=f"ao{h}_{qt}") for qt in range(NQ)]
            for h in range(HL)
        ]

        def v_copy(t, vps):
            for h4 in range(HL):
                nc.any.tensor_copy(V_t[t][:, h4, 0:128], vps[:, 128 * h4 : 128 * (h4 + 1)])

        def qk_proj(h, mc, QT, KT):
            ms = slice(512 * mc, 512 * (mc + 1))
            qps = ps_big.tile([128, 512], F32, tag="big", name="qps")
            for k in range(NKT):
                nc.tensor.matmul(
                    qps, lhsT=wqkv_t[k][:, 128 * h : 128 * (h + 1)], rhs=ht_rhs(mc, k),
                    start=(k == 0), stop=(k == NKT - 1),
                )
            nc.vector.tensor_scalar_mul(QT[:, ms], qps, SCALE)
            kps = ps_big.tile([128, 512], F32, tag="big", name="kps")
            for k in range(NKT):
                nc.tensor.matmul(
                    kps, lhsT=wqkv_t[k][:, 512 + 128 * h : 512 + 128 * (h + 1)], rhs=ht_rhs(mc, k),
                    start=(k == 0), stop=(k == NKT - 1),
                )
            nc.vector.tensor_copy(KT[:, ms], kps)

        def v_proj(t):
            vps = ps_big.tile([128, 512], F32, tag="big", name="vps")
            for k in range(NKT):
                nc.tensor.matmul(
                    vps, lhsT=ht_col(t, k), rhs=wqkv_t[k][:, 1024:1536],
                    start=(k == 0), stop=(k == NKT - 1),
                )
            v_copy(t, vps)

        def attention(h, QT, KT):
            for qt in range(NQ):
                t0 = max(0, 128 * qt - 128)
                t1 = min(S, 128 * qt + 256)
                nch = (t1 - t0) // 128
                # scores^T: one accumulation group, blocks at distinct cols
                scps = ps_sc.tile([128, 384], F32, tag="sc", name="scps")
                for b in range(nch):
                    nc.tensor.matmul(
                        scps[:, 128 * b : 128 * (b + 1)],
                        lhsT=KT[:, t0 + 128 * b : t0 + 128 * (b + 1)],
                        rhs=QT[:, 128 * qt : 128 * (qt + 1)],
                        start=(b == 0), stop=(b == nch - 1),
                        skip_group_check=True,
                    )
                # scores are O(+-8): exp needs no max subtraction (softmax is
                # shift-invariant; fp32 exp is safe here)
                p = work.tile([128, 384], BF, tag="p", name="p")
                nc.scalar.activation(
                    p[:, 0 : 128 * nch], scps[:, 0 : 128 * nch],
                    mybir.ActivationFunctionType.Exp, bias=0.0, scale=1.0,
                )
                # band corners are invalid: zero them in P^T instead of
                # adding -inf masks pre-exp
                if qt > 0:
                    nc.gpsimd.memset(p[0:64, 64:128], 0.0)
                if qt < NQ - 1:
                    nc.gpsimd.memset(p[64:128, 128 * (nch - 1) : 128 * (nch - 1) + 64], 0.0)
                aops = ps_ao.tile([128, 129], F32, tag="ao", name="aops")
                for b in range(nch):
                    tt = t0 // 128 + b
                    nc.tensor.matmul(
                        aops, lhsT=p[:, 128 * b : 128 * (b + 1)], rhs=V_t[tt][:, h, :],
                        start=(b == 0), stop=(b == nch - 1),
                    )
                rcp = stats.tile([128, 1], F32, tag="rcp", name="rcp")
                nc.vector.reciprocal(rcp, aops[:, 128:129])
                ao_sb = work.tile([128, 128], BF, tag="aosb", name="ao_sb")
                nc.vector.tensor_scalar_mul(ao_sb, aops[:, 0:128], rcp)
                nc.sync.dma_start_transpose(out=aot[h][qt], in_=ao_sb)
                if h == HL - 1 and qt >= WO_LAG:
                    _emit_wo(nc, ps_big, osb_pool, aot, wo_t, out, qt - WO_LAG)

        # ---- head 0: chase phase (V t0-3 + Q/K mc0 consume k-slices in
        # arrival order), then mc1-3 interleaved with V by quarter
        QT0 = qk.tile([128, S], BF, tag="q", name="qt0")
        KT0 = qk.tile([128, S], BF, tag="k", name="kt0")
        vps_c = [ps_big.tile([128, 512], F32, tag="big", name=f"vpsc{t}") for t in range(4)]
        qps_c = ps_big.tile([128, 512], F32, tag="big", name="qpsc")
        kps_c = ps_big.tile([128, 512], F32, tag="big", name="kpsc")
        for k in range(NKT):
            st, sp = (k == 0), (k == NKT - 1)
            nc.tensor.matmul(
                qps_c, lhsT=wqkv_t[k][:, 0:128], rhs=htq_t[0][k], start=st, stop=sp
            )
            nc.tensor.matmul(
                kps_c, lhsT=wqkv_t[k][:, 512:640], rhs=htq_t[0][k], start=st, stop=sp
            )
            for t in range(4):
                nc.tensor.matmul(
                    vps_c[t], lhsT=htq_t[0][k][:, 128 * t : 128 * (t + 1)],
                    rhs=wqkv_t[k][:, 1024:1536], start=st, stop=sp,
                )
        nc.vector.tensor_scalar_mul(QT0[:, 0:512], qps_c, SCALE)
        nc.vector.tensor_copy(KT0[:, 0:512], kps_c)
        for t in range(4):
            v_copy(t, vps_c[t])

        # quarter-paced: mc1 + V t4-7 chase htq q1, mc2 + V t8-11 chase q2,
        # mc3 + V t12-15 chase q3
        for mc in (1, 2, 3):
            qk_proj(0, mc, QT0, KT0)
            for t in range(4 * mc, 4 * mc + 4):
                v_proj(t)
        attention(0, QT0, KT0)

        for h in range(1, HL):
            QT = qk.tile([128, S], BF, tag="q", name=f"qt{h}")
            KT = qk.tile([128, S], BF, tag="k", name=f"kt{h}")
            for mc in range(4):
                qk_proj(h, mc, QT, KT)
            attention(h, QT, KT)
        for mt in range(NQ - WO_LAG, NQ):
            _emit_wo(nc, ps_big, osb_pool, aot, wo_t, out, mt)

    if not nc.is_finalized():
        nc.finalize()
    return nc


_NC = None


def _get_nc():
    global _NC
    if _NC is None:
        _NC = build()
    return _NC


def _in_maps(hidden_states, Wq, Wk, Wv, Wo):
    import ml_dtypes

    bf = ml_dtypes.bfloat16
    hs = np.asarray(hidden_states, dtype=np.float32)
    Wq = np.asarray(Wq, dtype=np.float32)
    Wk = np.asarray(Wk, dtype=np.float32)
    Wv = np.asarray(Wv, dtype=np.float32)
    Wo = np.asarray(Wo, dtype=np.float32)
    maps = []
    for c in range(8):
        b, g = divmod(c, 4)
        sl = slice(512 * g, 512 * (g + 1))
        hsT = hs[b].T  # [hid, seq]
        htq = np.concatenate([hsT[:, 512 * q : 512 * (q + 1)] for q in range(2)], axis=0)
        wqkv = np.concatenate([Wq[:, sl], Wk[:, sl], Wv[:, sl]], axis=1)
        maps.append(
            {
                "htq": np.ascontiguousarray(htq).astype(bf),
                "hth": np.ascontiguousarray(hsT[:, 1024:2048]).astype(bf),
                "wqkv": np.ascontiguousarray(wqkv).astype(bf),
                "wo": np.ascontiguousarray(Wo[sl, :]).astype(bf),
            }
        )
    return maps


def _gather(results):
    outs = [np.asarray(results[c]["out"]).astype(np.float32) for c in range(8)]
    return np.stack(
        [outs[0] + outs[1] + outs[2] + outs[3],
         outs[4] + outs[5] + outs[6] + outs[7]]
    )


def run(in_maps, trace=False, **kw):
    nc = _get_nc()
    return run_bass_kernel_spmd(nc, in_maps, core_ids=list(range(8)), trace=trace, **kw)


def kernel(hidden_states, Wq, Wk, Wv, Wo):
    maps = _in_maps(hidden_states, Wq, Wk, Wv, Wo)
    res = run(maps)
    return _gather(res.results)
